# revision 4
# baseline (speedup 1.0000x reference)
"""VQ codebook kernel (quant_conv linear -> cdist argmin -> gather + loss)
for 8 Trainium2 NeuronCores, data-parallel over the batch dim.

Numerical strategy: all matmuls run as 3-term fp16 hi/lo splits on the
TensorEngine (hi*hi + lo*hi + hi*lo), which is both faster than native fp32
(3 cyc/row vs 4) and more accurate on HW (~3e-6 abs score error vs the
~2^-17 of the HW fp32 path). The min top-2 distance gap for this problem is
~1.1e-5, so the argmin matches the fp32 reference exactly.

The -0.5*||e_k||^2 bias rides the PSUM accumulation as one extra K=2 matmul
(ones[2,128]^T @ [beta_hi; beta_lo]) so scores come out of PSUM already
biased; argmax value+index via DVE max8/max_index on two 4096-wide halves.
emb_loss uses sum((zq-h)^2) == sum_i(||h_i||^2 - 2*s*_i) so only Sum(h^2)
and Sum(s*) partials leave the device.
"""

import functools
from contextlib import ExitStack

import numpy as np

P = 128
B, T, ZC, D, K = 8, 4096, 512, 256, 8192
NCORES = 8
TOK = B * T // NCORES          # tokens per core = 4096
NT = TOK // P                  # token tiles per core = 32
NG = TOK // 512                # token groups for the h matmul = 8
NCH = K // 512                 # 512-code chunks = 16
SCALE = 32.0                   # operand pre-scale so f16 "lo" parts stay normal
BETA = 0.25


def _split_f16(a32):
    hi = a32.astype(np.float16)
    lo = (a32 - hi.astype(np.float32)).astype(np.float16)
    return hi, lo


@functools.lru_cache(maxsize=1)
def _build_graph():
    import concourse.bass as bass
    import concourse.tile as tile
    from concourse import bacc, mybir

    f32 = mybir.dt.float32
    f16 = mybir.dt.float16
    i32 = mybir.dt.int32
    u32 = mybir.dt.uint32
    Alu = mybir.AluOpType
    Act = mybir.ActivationFunctionType
    X = mybir.AxisListType.X

    nc = bacc.Bacc(
        "TRN2",
        target_bir_lowering=False,
        debug=False,
        enable_asserts=True,
        num_devices=NCORES,
    )

    # ---- DRAM I/O (per-core shard layouts, already munged on host) ----
    xt_hi = nc.dram_tensor("xt_hi", [P, 4, TOK], f16, kind="ExternalInput").ap()
    xt_lo = nc.dram_tensor("xt_lo", [P, 4, TOK], f16, kind="ExternalInput").ap()
    w_hi = nc.dram_tensor("w_hi", [P, 4, D], f16, kind="ExternalInput").ap()
    w_lo = nc.dram_tensor("w_lo", [P, 4, D], f16, kind="ExternalInput").ap()
    bvec = nc.dram_tensor("bvec", [P, 2], f32, kind="ExternalInput").ap()
    cb_hi = nc.dram_tensor("cb_hi", [P, 2, K], f16, kind="ExternalInput").ap()
    cb_lo = nc.dram_tensor("cb_lo", [P, 2, K], f16, kind="ExternalInput").ap()
    bb = nc.dram_tensor("bb", [2, K], f16, kind="ExternalInput").ap()
    ones2 = nc.dram_tensor("ones2", [2, P], f16, kind="ExternalInput").ap()
    cbrows = nc.dram_tensor("cbrows", [K, D], f32, kind="ExternalInput").ap()
    quant = nc.dram_tensor("quant", [TOK, D], f32, kind="ExternalOutput").ap()
    lpart = nc.dram_tensor("lpart", [P, 2], f32, kind="ExternalOutput").ap()

    with tile.TileContext(nc) as tc, ExitStack() as ctx:
        const = ctx.enter_context(tc.tile_pool(name="const", bufs=1))
        accum = ctx.enter_context(tc.tile_pool(name="accum", bufs=1))
        hbuf = ctx.enter_context(tc.tile_pool(name="hbuf", bufs=1))
        xin = ctx.enter_context(tc.tile_pool(name="xin", bufs=2))
        temps = ctx.enter_context(tc.tile_pool(name="temps", bufs=2))
        scores = ctx.enter_context(tc.tile_pool(name="scores", bufs=2))
        small = ctx.enter_context(tc.tile_pool(name="small", bufs=8))
        zqp = ctx.enter_context(tc.tile_pool(name="zqp", bufs=3))

        # resident constants
        w_hi_sb = const.tile([P, 4, D], f16)
        nc.sync.dma_start(w_hi_sb[:], w_hi[:])
        w_lo_sb = const.tile([P, 4, D], f16)
        nc.sync.dma_start(w_lo_sb[:], w_lo[:])
        bvec_sb = const.tile([P, 2], f32)
        nc.sync.dma_start(bvec_sb[:], bvec[:])
        cb_hi_sb = const.tile([P, 2, K], f16)
        nc.sync.dma_start(cb_hi_sb[:], cb_hi[:])
        cb_lo_sb = const.tile([P, 2, K], f16)
        nc.sync.dma_start(cb_lo_sb[:], cb_lo[:])
        bb_sb = const.tile([2, K], f16)
        nc.sync.dma_start(bb_sb[:], bb[:])
        ones2_sb = const.tile([2, P], f16)
        nc.sync.dma_start(ones2_sb[:], ones2[:])

        h_hi_sb = hbuf.tile([P, 2, TOK], f16)
        h_lo_sb = hbuf.tile([P, 2, TOK], f16)
        sq_cols = accum.tile([P, 2 * NG], f32)
        sstar_cols = accum.tile([P, NT], f32)

        # ---- Phase A: hT = (W32^T @ x^T)/32 + b, fp16 3-term split ----
        with tc.tile_pool(name="psA", bufs=2, space="PSUM") as psA:
            for g in range(NG):
                xt_hi_t = xin.tile([P, 4, 512], f16, tag="xhi")
                nc.sync.dma_start(xt_hi_t[:], xt_hi[:, :, g * 512:(g + 1) * 512])
                xt_lo_t = xin.tile([P, 4, 512], f16, tag="xlo")
                nc.sync.dma_start(xt_lo_t[:], xt_lo[:, :, g * 512:(g + 1) * 512])
                for dc in range(2):
                    ps = psA.tile([P, 512], f32)
                    mm = 0
                    for (ws, xs) in ((w_hi_sb, xt_hi_t), (w_hi_sb, xt_lo_t),
                                     (w_lo_sb, xt_hi_t)):
                        for kz in range(4):
                            nc.tensor.matmul(
                                out=ps[:],
                                lhsT=ws[:, kz, dc * P:(dc + 1) * P],
                                rhs=xs[:, kz, :],
                                start=(mm == 0),
                                stop=(mm == 11),
                            )
                            mm += 1
                    h32t = temps.tile([P, 512], f32, tag="h32")
                    nc.scalar.activation(h32t[:], ps[:], Act.Identity,
                                         bias=bvec_sb[:, dc:dc + 1],
                                         scale=1.0 / SCALE)
                    sqt = temps.tile([P, 512], f32, tag="sq")
                    nc.scalar.activation(sqt[:], ps[:], Act.Square,
                                         bias=bvec_sb[:, dc:dc + 1],
                                         scale=1.0 / SCALE)
                    nc.vector.reduce_sum(sq_cols[:, 2 * g + dc:2 * g + dc + 1],
                                         sqt[:], axis=X)
                    hslice = (slice(None), dc, slice(g * 512, (g + 1) * 512))
                    nc.vector.tensor_copy(h_hi_sb[hslice], h32t[:])
                    hi32 = temps.tile([P, 512], f32, tag="hi32")
                    nc.vector.tensor_copy(hi32[:], h_hi_sb[hslice])
                    rt = temps.tile([P, 512], f32, tag="rt")
                    nc.vector.tensor_sub(rt[:], h32t[:], hi32[:])
                    nc.vector.tensor_copy(h_lo_sb[hslice], rt[:])

        # ---- Phase B: biased scores, argmax, gather ----
        with tc.tile_pool(name="psB", bufs=2, space="PSUM") as psB:
            for t in range(NT):
                halves = []
                for q in range(4):  # 4 chunk-groups of 4x512 codes
                    ps = psB.tile([P, 2048], f32)
                    for c in range(4):
                        n = q * 4 + c
                        sl = ps[:, c * 512:(c + 1) * 512]
                        # bias matmul: ones[2,128]^T @ [beta_hi;beta_lo]
                        nc.tensor.matmul(
                            out=sl, lhsT=ones2_sb[:, :],
                            rhs=bb_sb[:, n * 512:(n + 1) * 512],
                            start=True, stop=False,
                        )
                        mm = 0
                        for (hs, cs) in ((h_hi_sb, cb_hi_sb), (h_lo_sb, cb_hi_sb),
                                         (h_hi_sb, cb_lo_sb)):
                            for dc in range(2):
                                nc.tensor.matmul(
                                    out=sl,
                                    lhsT=hs[:, dc, t * P:(t + 1) * P],
                                    rhs=cs[:, dc, n * 512:(n + 1) * 512],
                                    start=False,
                                    stop=(mm == 5),
                                )
                                mm += 1
                    if q % 2 == 0:
                        sc = scores.tile([P, 4096], f32, tag="sc")
                        halves.append(sc)
                    nc.scalar.activation(
                        sc[:, (q % 2) * 2048:(q % 2 + 1) * 2048], ps[:],
                        Act.Copy, scale=1.0 / SCALE)

                m8s, i8s = [], []
                for h in range(2):
                    m8 = small.tile([P, 8], f32, tag="m8")
                    nc.vector.max(m8[:], halves[h][:])
                    i8 = small.tile([P, 8], u32, tag="i8")
                    nc.vector.max_index(i8[:], m8[:], halves[h][:])
                    m8s.append(m8)
                    i8s.append(i8)

                # merge halves: s* = max(m0,m1); idx = m0>=m1 ? i0 : i1+4096
                i0f = small.tile([P, 1], f32, tag="i0f")
                nc.vector.tensor_copy(i0f[:], i8s[0][:, 0:1])
                i1f = small.tile([P, 1], f32, tag="i1f")
                nc.vector.tensor_copy(i1f[:], i8s[1][:, 0:1])
                nc.vector.tensor_max(sstar_cols[:, t:t + 1],
                                     m8s[0][:, 0:1], m8s[1][:, 0:1])
                sel = small.tile([P, 1], f32, tag="sel")
                nc.vector.tensor_tensor(out=sel[:], in0=m8s[0][:, 0:1],
                                        in1=m8s[1][:, 0:1], op=Alu.is_ge)
                dd = small.tile([P, 1], f32, tag="dd")
                nc.vector.tensor_sub(dd[:], i0f[:], i1f[:])
                dd2 = small.tile([P, 1], f32, tag="dd2")
                nc.vector.tensor_scalar_add(dd2[:], dd[:], -4096.0)
                md = small.tile([P, 1], f32, tag="md")
                nc.vector.tensor_mul(md[:], sel[:], dd2[:])
                i1p = small.tile([P, 1], f32, tag="i1p")
                nc.vector.tensor_scalar_add(i1p[:], i1f[:], 4096.0)
                idxf = small.tile([P, 1], f32, tag="idxf")
                nc.vector.tensor_add(idxf[:], i1p[:], md[:])
                idx_i = small.tile([P, 1], i32, tag="idxi")
                nc.vector.tensor_copy(idx_i[:], idxf[:])

                zq = zqp.tile([P, D], f32, tag="zq")
                nc.gpsimd.indirect_dma_start(
                    out=zq[:], out_offset=None, in_=cbrows[:],
                    in_offset=bass.IndirectOffsetOnAxis(ap=idx_i[:, :1], axis=0),
                )
                nc.sync.dma_start(quant[t * P:(t + 1) * P, :], zq[:])

        # ---- loss partials ----
        lp = small.tile([P, 2], f32, tag="lp")
        nc.vector.reduce_sum(lp[:, 0:1], sq_cols[:], axis=X)
        nc.vector.reduce_sum(lp[:, 1:2], sstar_cols[:], axis=X)
        nc.sync.dma_start(lpart[:], lp[:])

    nc.compile()
    return nc


def _prep_inputs(x, W, b, codebook):
    """Host-side shard + layout prep. Returns per-core input maps."""
    x = np.asarray(x, dtype=np.float32)
    W = np.asarray(W, dtype=np.float32)
    b = np.asarray(b, dtype=np.float32)
    cb = np.asarray(codebook, dtype=np.float32)

    Ws = W * np.float32(SCALE)                      # [512, 256]
    w_hi, w_lo = _split_f16(Ws)
    w_hi = np.ascontiguousarray(w_hi.reshape(4, P, D).transpose(1, 0, 2))
    w_lo = np.ascontiguousarray(w_lo.reshape(4, P, D).transpose(1, 0, 2))
    bvec = np.ascontiguousarray(b.reshape(2, P).T)  # [128, 2]

    cbs = np.ascontiguousarray(cb.T) * np.float32(SCALE)   # [256, 8192]
    cb_hi, cb_lo = _split_f16(cbs)
    cb_hi = np.ascontiguousarray(cb_hi.reshape(2, P, K).transpose(1, 0, 2))
    cb_lo = np.ascontiguousarray(cb_lo.reshape(2, P, K).transpose(1, 0, 2))

    beta = (-0.5 * np.sum(cb.astype(np.float64) ** 2, axis=1)).astype(np.float32)
    bb_hi, bb_lo = _split_f16(beta * np.float32(SCALE))
    bb = np.stack([bb_hi, bb_lo])                   # [2, 8192]
    ones2 = np.ones((2, P), dtype=np.float16)

    xs = x.reshape(B, T, ZC)                        # squeeze dim 2
    in_maps = []
    for i in range(NCORES):
        xT = np.ascontiguousarray(xs[i].T)          # [512, 4096]
        xt_hi, xt_lo = _split_f16(xT)
        xt_hi = np.ascontiguousarray(xt_hi.reshape(4, P, TOK).transpose(1, 0, 2))
        xt_lo = np.ascontiguousarray(xt_lo.reshape(4, P, TOK).transpose(1, 0, 2))
        in_maps.append({
            "xt_hi": xt_hi, "xt_lo": xt_lo,
            "w_hi": w_hi, "w_lo": w_lo, "bvec": bvec,
            "cb_hi": cb_hi, "cb_lo": cb_lo, "bb": bb, "ones2": ones2,
            "cbrows": cb,
        })
    return in_maps


def kernel(x, W, b, codebook):
    from concourse.bass_utils import run_bass_kernel_spmd

    nc = _build_graph()
    in_maps = _prep_inputs(x, W, b, codebook)
    res = run_bass_kernel_spmd(nc, in_maps, core_ids=list(range(NCORES)))
    global _last_results
    _last_results = res

    quant = np.empty((B, T, D, 1), dtype=np.float32)
    tot_sq = 0.0
    tot_s = 0.0
    for i in range(NCORES):
        out = res.results[i]
        quant[i, :, :, 0] = out["quant"]
        lp = out["lpart"].astype(np.float64)
        tot_sq += lp[:, 0].sum()
        tot_s += lp[:, 1].sum()
    mean_sq = (tot_sq - 2.0 * tot_s) / (B * T * D)
    emb_loss = np.float32((1.0 + BETA) * mean_sq)
    return quant, emb_loss


# revision 8
# speedup vs baseline: 4515.5766x; 4515.5766x over previous
"""VQ codebook kernel (quant_conv linear -> cdist argmin -> gather + loss)
for 8 Trainium2 NeuronCores, data-parallel over the batch dim.

Numerical strategy: all matmuls run as 3-term fp16 hi/lo splits on the
TensorEngine (hi*hi + lo*hi + hi*lo), which is both faster than native fp32
(3 cyc/row vs 4) and more accurate on HW (~3e-6 abs score error vs the
~2^-17 of the HW fp32 path). The min top-2 distance gap for this problem is
~1.1e-5, so the argmin matches the fp32 reference exactly.

The -0.5*||e_k||^2 bias rides the PSUM accumulation as one extra K=2 matmul
(ones[2,128]^T @ [beta_hi; beta_lo]) so scores come out of PSUM already
biased; argmax value+index via DVE max8/max_index on two 4096-wide halves.
emb_loss uses sum((zq-h)^2) == sum_i(||h_i||^2 - 2*s*_i) so only Sum(h^2)
and Sum(s*) partials leave the device.
"""

import functools
from contextlib import ExitStack

import numpy as np

P = 128
B, T, ZC, D, K = 8, 4096, 512, 256, 8192
NCORES = 8
TOK = B * T // NCORES          # tokens per core = 4096
NT = TOK // P                  # token tiles per core = 32
NG = TOK // 512                # token groups for the h matmul = 8
NCH = K // 512                 # 512-code chunks = 16
SCALE = 32.0                   # operand pre-scale so f16 "lo" parts stay normal
BETA = 0.25


def _split_f16(a32):
    hi = a32.astype(np.float16)
    lo = (a32 - hi.astype(np.float32)).astype(np.float16)
    return hi, lo


@functools.lru_cache(maxsize=2)
def _build_graph(reps: int = 1):
    """Build the SPMD graph. reps>1 repeats the whole computation inside one
    NEFF (same I/O) — used only for on-device timing measurements."""
    import concourse.bass as bass
    import concourse.tile as tile
    from concourse import bacc, mybir

    f32 = mybir.dt.float32
    f16 = mybir.dt.float16
    i32 = mybir.dt.int32
    u32 = mybir.dt.uint32
    Alu = mybir.AluOpType
    Act = mybir.ActivationFunctionType
    X = mybir.AxisListType.X

    nc = bacc.Bacc(
        "TRN2",
        target_bir_lowering=False,
        debug=False,
        enable_asserts=True,
        num_devices=NCORES,
    )

    # ---- DRAM I/O (per-core shard layouts, already munged on host) ----
    xt_hi = nc.dram_tensor("xt_hi", [P, 4, TOK], f16, kind="ExternalInput").ap()
    xt_lo = nc.dram_tensor("xt_lo", [P, 4, TOK], f16, kind="ExternalInput").ap()
    w_hi = nc.dram_tensor("w_hi", [P, 4, D], f16, kind="ExternalInput").ap()
    w_lo = nc.dram_tensor("w_lo", [P, 4, D], f16, kind="ExternalInput").ap()
    bvec = nc.dram_tensor("bvec", [P, 2], f32, kind="ExternalInput").ap()
    cb_hi = nc.dram_tensor("cb_hi", [P, 2, K], f16, kind="ExternalInput").ap()
    cb_lo = nc.dram_tensor("cb_lo", [P, 2, K], f16, kind="ExternalInput").ap()
    bb = nc.dram_tensor("bb", [2, K], f16, kind="ExternalInput").ap()
    ones2 = nc.dram_tensor("ones2", [2, P], f16, kind="ExternalInput").ap()
    cbrows = nc.dram_tensor("cbrows", [K, D], f32, kind="ExternalInput").ap()
    quant = nc.dram_tensor("quant", [TOK, D], f32, kind="ExternalOutput").ap()
    lpart = nc.dram_tensor("lpart", [P, 2], f32, kind="ExternalOutput").ap()

    with tile.TileContext(nc) as tc, ExitStack() as ctx:
        const = ctx.enter_context(tc.tile_pool(name="const", bufs=1))
        accum = ctx.enter_context(tc.tile_pool(name="accum", bufs=1))
        hbuf = ctx.enter_context(tc.tile_pool(name="hbuf", bufs=1))
        xin = ctx.enter_context(tc.tile_pool(name="xin", bufs=2))
        temps = ctx.enter_context(tc.tile_pool(name="temps", bufs=2))
        scores = ctx.enter_context(tc.tile_pool(name="scores", bufs=2))
        small = ctx.enter_context(tc.tile_pool(name="small", bufs=8))
        zqp = ctx.enter_context(tc.tile_pool(name="zqp", bufs=3))

        # resident constants
        w_hi_sb = const.tile([P, 4, D], f16)
        nc.sync.dma_start(w_hi_sb[:], w_hi[:])
        w_lo_sb = const.tile([P, 4, D], f16)
        nc.sync.dma_start(w_lo_sb[:], w_lo[:])
        bvec_sb = const.tile([P, 2], f32)
        nc.sync.dma_start(bvec_sb[:], bvec[:])
        cb_hi_sb = const.tile([P, 2, K], f16)
        nc.sync.dma_start(cb_hi_sb[:], cb_hi[:])
        cb_lo_sb = const.tile([P, 2, K], f16)
        nc.sync.dma_start(cb_lo_sb[:], cb_lo[:])
        bb_sb = const.tile([2, K], f16)
        nc.sync.dma_start(bb_sb[:], bb[:])
        ones2_sb = const.tile([2, P], f16)
        nc.sync.dma_start(ones2_sb[:], ones2[:])

        h_hi_sb = hbuf.tile([P, 2, TOK], f16)
        h_lo_sb = hbuf.tile([P, 2, TOK], f16)
        sq_cols = accum.tile([P, 2 * NG], f32)
        sstar_cols = accum.tile([P, NT], f32)

        for _rep in range(reps):
            _run_body(nc, tc, locals())

    nc.compile()
    return nc


def _run_body(nc, tc, env):
    import concourse.bass as bass
    from concourse import mybir

    f32 = mybir.dt.float32
    f16 = mybir.dt.float16
    i32 = mybir.dt.int32
    u32 = mybir.dt.uint32
    Alu = mybir.AluOpType
    Act = mybir.ActivationFunctionType
    X = mybir.AxisListType.X
    (xt_hi, xt_lo, bvec, quant, lpart, xin, temps, scores, small, zqp,
     w_hi_sb, w_lo_sb, bvec_sb, cb_hi_sb, cb_lo_sb, bb_sb, ones2_sb,
     h_hi_sb, h_lo_sb, sq_cols, sstar_cols, cbrows) = (
        env["xt_hi"], env["xt_lo"], env["bvec"], env["quant"], env["lpart"],
        env["xin"], env["temps"], env["scores"], env["small"], env["zqp"],
        env["w_hi_sb"], env["w_lo_sb"], env["bvec_sb"], env["cb_hi_sb"],
        env["cb_lo_sb"], env["bb_sb"], env["ones2_sb"], env["h_hi_sb"],
        env["h_lo_sb"], env["sq_cols"], env["sstar_cols"], env["cbrows"])

    if True:
        # ---- Phase A: hT = (W32^T @ x^T)/32 + b, fp16 3-term split ----
        with tc.tile_pool(name="psA", bufs=2, space="PSUM") as psA:
            for g in range(NG):
                xt_hi_t = xin.tile([P, 4, 512], f16, tag="xhi")
                nc.sync.dma_start(xt_hi_t[:], xt_hi[:, :, g * 512:(g + 1) * 512])
                xt_lo_t = xin.tile([P, 4, 512], f16, tag="xlo")
                nc.sync.dma_start(xt_lo_t[:], xt_lo[:, :, g * 512:(g + 1) * 512])
                for dc in range(2):
                    ps = psA.tile([P, 512], f32)
                    mm = 0
                    for (ws, xs) in ((w_hi_sb, xt_hi_t), (w_hi_sb, xt_lo_t),
                                     (w_lo_sb, xt_hi_t)):
                        for kz in range(4):
                            nc.tensor.matmul(
                                out=ps[:],
                                lhsT=ws[:, kz, dc * P:(dc + 1) * P],
                                rhs=xs[:, kz, :],
                                start=(mm == 0),
                                stop=(mm == 11),
                            )
                            mm += 1
                    h32t = temps.tile([P, 512], f32, tag="h32")
                    nc.scalar.activation(h32t[:], ps[:], Act.Identity,
                                         bias=bvec_sb[:, dc:dc + 1],
                                         scale=1.0 / SCALE)
                    sqt = temps.tile([P, 512], f32, tag="sq")
                    nc.scalar.activation(sqt[:], ps[:], Act.Square,
                                         bias=bvec_sb[:, dc:dc + 1],
                                         scale=1.0 / SCALE)
                    nc.vector.reduce_sum(sq_cols[:, 2 * g + dc:2 * g + dc + 1],
                                         sqt[:], axis=X)
                    hslice = (slice(None), dc, slice(g * 512, (g + 1) * 512))
                    nc.vector.tensor_copy(h_hi_sb[hslice], h32t[:])
                    hi32 = temps.tile([P, 512], f32, tag="hi32")
                    nc.vector.tensor_copy(hi32[:], h_hi_sb[hslice])
                    rt = temps.tile([P, 512], f32, tag="rt")
                    nc.vector.tensor_sub(rt[:], h32t[:], hi32[:])
                    nc.vector.tensor_copy(h_lo_sb[hslice], rt[:])

        # ---- Phase B: biased scores, argmax, gather ----
        with tc.tile_pool(name="psB", bufs=2, space="PSUM") as psB:
            for t in range(NT):
                halves = []
                for q in range(4):  # 4 chunk-groups of 4x512 codes
                    ps = psB.tile([P, 2048], f32)
                    for c in range(4):
                        n = q * 4 + c
                        sl = ps[:, c * 512:(c + 1) * 512]
                        # bias matmul: ones[2,128]^T @ [beta_hi;beta_lo]
                        nc.tensor.matmul(
                            out=sl, lhsT=ones2_sb[:, :],
                            rhs=bb_sb[:, n * 512:(n + 1) * 512],
                            start=True, stop=False,
                        )
                        mm = 0
                        for (hs, cs) in ((h_hi_sb, cb_hi_sb), (h_lo_sb, cb_hi_sb),
                                         (h_hi_sb, cb_lo_sb)):
                            for dc in range(2):
                                nc.tensor.matmul(
                                    out=sl,
                                    lhsT=hs[:, dc, t * P:(t + 1) * P],
                                    rhs=cs[:, dc, n * 512:(n + 1) * 512],
                                    start=False,
                                    stop=(mm == 5),
                                )
                                mm += 1
                    if q % 2 == 0:
                        sc = scores.tile([P, 4096], f32, tag="sc")
                        halves.append(sc)
                    nc.scalar.activation(
                        sc[:, (q % 2) * 2048:(q % 2 + 1) * 2048], ps[:],
                        Act.Copy, scale=1.0 / SCALE)

                m8s, i8s = [], []
                for h in range(2):
                    m8 = small.tile([P, 8], f32, tag="m8")
                    nc.vector.max(m8[:], halves[h][:])
                    i8 = small.tile([P, 8], u32, tag="i8")
                    nc.vector.max_index(i8[:], m8[:], halves[h][:])
                    m8s.append(m8)
                    i8s.append(i8)

                # merge halves: s* = max(m0,m1); idx = m0>=m1 ? i0 : i1+4096
                i0f = small.tile([P, 1], f32, tag="i0f")
                nc.vector.tensor_copy(i0f[:], i8s[0][:, 0:1])
                i1f = small.tile([P, 1], f32, tag="i1f")
                nc.vector.tensor_copy(i1f[:], i8s[1][:, 0:1])
                nc.vector.tensor_max(sstar_cols[:, t:t + 1],
                                     m8s[0][:, 0:1], m8s[1][:, 0:1])
                sel = small.tile([P, 1], f32, tag="sel")
                nc.vector.tensor_tensor(out=sel[:], in0=m8s[0][:, 0:1],
                                        in1=m8s[1][:, 0:1], op=Alu.is_ge)
                dd = small.tile([P, 1], f32, tag="dd")
                nc.vector.tensor_sub(dd[:], i0f[:], i1f[:])
                dd2 = small.tile([P, 1], f32, tag="dd2")
                nc.vector.tensor_scalar_add(dd2[:], dd[:], -4096.0)
                md = small.tile([P, 1], f32, tag="md")
                nc.vector.tensor_mul(md[:], sel[:], dd2[:])
                i1p = small.tile([P, 1], f32, tag="i1p")
                nc.vector.tensor_scalar_add(i1p[:], i1f[:], 4096.0)
                idxf = small.tile([P, 1], f32, tag="idxf")
                nc.vector.tensor_add(idxf[:], i1p[:], md[:])
                idx_i = small.tile([P, 1], i32, tag="idxi")
                nc.vector.tensor_copy(idx_i[:], idxf[:])

                zq = zqp.tile([P, D], f32, tag="zq")
                nc.gpsimd.indirect_dma_start(
                    out=zq[:], out_offset=None, in_=cbrows[:],
                    in_offset=bass.IndirectOffsetOnAxis(ap=idx_i[:, :1], axis=0),
                )
                nc.sync.dma_start(quant[t * P:(t + 1) * P, :], zq[:])

        # ---- loss partials ----
        lp = small.tile([P, 2], f32, tag="lp")
        nc.vector.reduce_sum(lp[:, 0:1], sq_cols[:], axis=X)
        nc.vector.reduce_sum(lp[:, 1:2], sstar_cols[:], axis=X)
        nc.sync.dma_start(lpart[:], lp[:])


def _prep_inputs(x, W, b, codebook):
    """Host-side shard + layout prep. Returns per-core input maps."""
    x = np.asarray(x, dtype=np.float32)
    W = np.asarray(W, dtype=np.float32)
    b = np.asarray(b, dtype=np.float32)
    cb = np.asarray(codebook, dtype=np.float32)

    Ws = W * np.float32(SCALE)                      # [512, 256]
    w_hi, w_lo = _split_f16(Ws)
    w_hi = np.ascontiguousarray(w_hi.reshape(4, P, D).transpose(1, 0, 2))
    w_lo = np.ascontiguousarray(w_lo.reshape(4, P, D).transpose(1, 0, 2))
    bvec = np.ascontiguousarray(b.reshape(2, P).T)  # [128, 2]

    cbs = np.ascontiguousarray(cb.T) * np.float32(SCALE)   # [256, 8192]
    cb_hi, cb_lo = _split_f16(cbs)
    cb_hi = np.ascontiguousarray(cb_hi.reshape(2, P, K).transpose(1, 0, 2))
    cb_lo = np.ascontiguousarray(cb_lo.reshape(2, P, K).transpose(1, 0, 2))

    beta = (-0.5 * np.sum(cb.astype(np.float64) ** 2, axis=1)).astype(np.float32)
    bb_hi, bb_lo = _split_f16(beta * np.float32(SCALE))
    bb = np.stack([bb_hi, bb_lo])                   # [2, 8192]
    ones2 = np.ones((2, P), dtype=np.float16)

    xs = x.reshape(B, T, ZC)                        # squeeze dim 2
    in_maps = []
    for i in range(NCORES):
        xT = np.ascontiguousarray(xs[i].T)          # [512, 4096]
        xt_hi, xt_lo = _split_f16(xT)
        xt_hi = np.ascontiguousarray(xt_hi.reshape(4, P, TOK).transpose(1, 0, 2))
        xt_lo = np.ascontiguousarray(xt_lo.reshape(4, P, TOK).transpose(1, 0, 2))
        in_maps.append({
            "xt_hi": xt_hi, "xt_lo": xt_lo,
            "w_hi": w_hi, "w_lo": w_lo, "bvec": bvec,
            "cb_hi": cb_hi, "cb_lo": cb_lo, "bb": bb, "ones2": ones2,
            "cbrows": cb,
        })
    return in_maps


def kernel(x, W, b, codebook):
    from concourse.bass_utils import run_bass_kernel_spmd

    nc = _build_graph()
    in_maps = _prep_inputs(x, W, b, codebook)
    res = run_bass_kernel_spmd(nc, in_maps, core_ids=list(range(NCORES)))
    global _last_results
    _last_results = res

    quant = np.empty((B, T, D, 1), dtype=np.float32)
    tot_sq = 0.0
    tot_s = 0.0
    for i in range(NCORES):
        out = res.results[i]
        quant[i, :, :, 0] = out["quant"]
        lp = out["lpart"].astype(np.float64)
        tot_sq += lp[:, 0].sum()
        tot_s += lp[:, 1].sum()
    mean_sq = (tot_sq - 2.0 * tot_s) / (B * T * D)
    emb_loss = np.float32((1.0 + BETA) * mean_sq)
    return quant, emb_loss
